# revision 5
# baseline (speedup 1.0000x reference)
"""CRF Viterbi decode kernel for Trainium2 (8 NeuronCores, data-parallel).

Device computes the forward max-plus scan (alpha) exactly; host does the
O(B*S*T) backtrack from alpha, which is bitwise-identical to the reference's
backpointer path.

Per core (64 batches), two independent 32-batch "chains" interleave so PE
matmuls of one chain overlap DVE reductions of the other:
  partition p = q*32 + b   (q in [0,4) tag-quarter, b in [0,32) batch)
  alpha slot s: [128, 16] tile region, value alpha_s[b, 16q+n]
  X psum [128, 1024]: X[(q,b), prev*16+n] = alpha_{s-1}[b, prev] + trans[16q+n, prev]
    - 4 selector matmuls (lhsT sel[qp], K=128) gather alpha_{s-1} (exact 0/1 weights)
    - 1 trans matmul (lhsT A4, K=4, rhs T4) accumulates the transition term
  DVE grouped reduce over prev (stride 16) -> m [128,16]; TT += feat -> alpha_s
"""
import os
import numpy as np

B, S, T = 512, 1024, 64
NCORES = 8
B_CORE = B // NCORES          # 64
B_CHAIN = B_CORE // 2         # 32
NQ, NN = 4, 16                # tag split: t = 16*q + n
CHUNK = 128                   # steps per feat/alpha chunk
NCHUNK = S // CHUNK
NEG = -10000.0
START_IX, END_IX = 62, 63

_CACHE = {}

LAST_RUN_INFO = {}


def _build_nc(trace=False):
    import concourse.bacc as bacc
    import concourse.tile as tile
    from concourse import mybir

    dt = mybir.dt.float32
    nc = bacc.Bacc()

    sel_d = [nc.declare_dram_parameter(f"sel{qp}", [128, 128], dt, isOutput=False)
             for qp in range(NQ)]
    A4_d = nc.declare_dram_parameter("A4", [NQ, 128], dt, isOutput=False)
    T4_d = nc.declare_dram_parameter("T4", [NQ, T * NN], dt, isOutput=False)
    feat_d = [nc.declare_dram_parameter(f"feat_c{c}", [128, S * NN], dt, isOutput=False)
              for c in range(2)]
    alpha_d = [nc.declare_dram_parameter(f"alpha_c{c}", [128, S * NN], dt, isOutput=True)
               for c in range(2)]

    with tile.TileContext(nc) as tc:
        with (
            tc.tile_pool(name="stage", bufs=1) as stage,
            tc.tile_pool(name="consts", bufs=1) as consts,
            tc.tile_pool(name="feat", bufs=2) as fpool,
            tc.tile_pool(name="alpha", bufs=2) as apool,
            tc.tile_pool(name="psum", bufs=1, space="PSUM") as pp,
        ):
            # ---- stage constants through DVE so PE waits only on the DVE sem
            sel_s = [stage.tile([128, 128], dt, name=f"sel_s{qp}", tag=f"sel_s{qp}") for qp in range(NQ)]
            A4_s = stage.tile([NQ, 128], dt, name="A4_s")
            T4_s = stage.tile([NQ, T * NN], dt, name="T4_s")
            for qp in range(NQ):
                nc.sync.dma_start(sel_s[qp][:], sel_d[qp][:])
            nc.sync.dma_start(A4_s[:], A4_d[:])
            nc.sync.dma_start(T4_s[:], T4_d[:])

            sel_t = [consts.tile([128, 128], dt, name=f"sel_t{qp}", tag=f"sel_t{qp}") for qp in range(NQ)]
            A4_t = consts.tile([NQ, 128], dt, name="A4_t")
            T4_t = consts.tile([NQ, T * NN], dt, name="T4_t")
            for qp in range(NQ):
                nc.vector.tensor_copy(sel_t[qp][:], sel_s[qp][:])
            nc.vector.tensor_copy(A4_t[:], A4_s[:])
            nc.vector.tensor_copy(T4_t[:], T4_s[:])

            # ---- HAM keepalive: fp32(HI/LO) matmuls don't count as PE-busy,
            # so the clock gate stays at 1.2GHz. A tiny bf16 matmul per step
            # keeps the activity window busy -> 2.4GHz for everything.
            bf = mybir.dt.bfloat16
            ka_w = consts.tile([32, 32], bf, name="ka_w")
            ka_x = consts.tile([32, 64], bf, name="ka_x")
            nc.vector.memset(ka_w[:], 0.0)
            nc.vector.memset(ka_x[:], 0.0)

            # ---- alpha_{-1} init: NEG everywhere, 0.0 at tag 62 = (q=3, n=14)
            init_t = [consts.tile([128, NN], dt, name=f"init{c}", tag=f"init{c}") for c in range(2)]
            for c in range(2):
                nc.vector.memset(init_t[c][:], NEG)
                nc.vector.memset(init_t[c][3 * 32:4 * 32, 14:15], 0.0)

            feat_tiles = [None, None]
            alpha_tiles = [None, None]
            prev_alpha = [None, None]   # (tile, col) of slot s-1

            ka_ps = pp.tile([32, 64], mybir.dt.float32, name="ka_ps", tag="ka")
            for s in range(S):
                nc.tensor.matmul(ka_ps[:], ka_w[:], ka_x[:], start=True, stop=True)
                k, j = divmod(s, CHUNK)
                if j == 0:
                    for c in range(2):
                        ft = fpool.tile([128, CHUNK * NN], dt, name=f"feat{c}_{k}", tag=f"feat{c}")
                        nc.sync.dma_start(
                            ft[:], feat_d[c][:, k * CHUNK * NN:(k + 1) * CHUNK * NN])
                        feat_tiles[c] = ft
                        alpha_tiles[c] = apool.tile([128, CHUNK * NN], dt, name=f"alpha{c}_{k}", tag=f"alpha{c}")
                for c in range(2):
                    at = alpha_tiles[c]
                    if s == 0:
                        slot_prev = init_t[c][:]
                    else:
                        pt, pj = prev_alpha[c]
                        slot_prev = pt[:, pj * NN:(pj + 1) * NN]
                    rhs = slot_prev.unsqueeze(2).broadcast_to([128, NN, NN])
                    X = pp.tile([128, T * NN], dt, name=f"X{c}_{s}", tag=f"X{c}")
                    # one start=True per PSUM bank: start clears has_written
                    # for the WHOLE bank, so the second mm in a bank must not
                    # re-clear (it overwrites where unset, which is its cols)
                    for qp in range(NQ):
                        nc.tensor.matmul(X[:, qp * 256:(qp + 1) * 256], sel_t[qp][:],
                                         rhs, start=(qp % 2 == 0), stop=False)
                    nc.tensor.matmul(X[:, 0:512], A4_t[:], T4_t[:, 0:512],
                                     start=False, stop=True)
                    nc.tensor.matmul(X[:, 512:1024], A4_t[:], T4_t[:, 512:1024],
                                     start=False, stop=True)
                    xin = X[:].rearrange("p (prev n) -> p n prev", n=NN)
                    m_ap = at[:, j * NN:(j + 1) * NN]
                    nc.vector.tensor_reduce(m_ap, xin, axis=mybir.AxisListType.X,
                                            op=mybir.AluOpType.max)
                    nc.vector.tensor_tensor(m_ap, m_ap,
                                            feat_tiles[c][:, j * NN:(j + 1) * NN],
                                            op=mybir.AluOpType.add)
                    prev_alpha[c] = (at, j)
                if j == CHUNK - 1:
                    for c in range(2):
                        nc.sync.dma_start(
                            alpha_d[c][:, k * CHUNK * NN:(k + 1) * CHUNK * NN],
                            alpha_tiles[c][:])

    nc.compile()
    return nc


def _host_inputs(feats, transitions):
    """Per-core input maps: selectors, A4, T4 layout of trans, per-chain feats."""
    feats = np.ascontiguousarray(np.asarray(feats, dtype=np.float32))
    trans = np.ascontiguousarray(np.asarray(transitions, dtype=np.float32))

    sel = [np.zeros((128, 128), dtype=np.float32) for _ in range(NQ)]
    for qp in range(NQ):
        for b in range(B_CHAIN):
            for q in range(NQ):
                sel[qp][qp * 32 + b, q * 32 + b] = 1.0
    A4 = np.zeros((NQ, 128), dtype=np.float32)
    for q in range(NQ):
        A4[q, q * 32:(q + 1) * 32] = 1.0
    # T4[q', prev*16+n] = trans[16*q'+n, prev]
    T4 = trans.reshape(NQ, NN, T).transpose(0, 2, 1).reshape(NQ, T * NN).copy()

    in_maps = []
    for i in range(NCORES):
        m = {f"sel{qp}": sel[qp] for qp in range(NQ)}
        m["A4"] = A4
        m["T4"] = T4
        for c in range(2):
            b0 = i * B_CORE + c * B_CHAIN
            blk = feats[b0:b0 + B_CHAIN]            # [32, S, 64]
            # row q*32+b, col s*16+n
            m[f"feat_c{c}"] = np.ascontiguousarray(
                blk.reshape(B_CHAIN, S, NQ, NN).transpose(2, 0, 1, 3)
                   .reshape(128, S * NN))
        in_maps.append(m)
    return in_maps, trans


def kernel(feats, transitions):
    from concourse.bass_utils import run_bass_kernel_spmd

    trace = bool(os.environ.get("CRF_KERNEL_TRACE"))
    key = "nc"
    if key not in _CACHE:
        _CACHE[key] = _build_nc()
    nc = _CACHE[key]

    in_maps, trans = _host_inputs(feats, transitions)
    res = run_bass_kernel_spmd(nc, in_maps, list(range(NCORES)), trace=trace)
    LAST_RUN_INFO["exec_time_ns"] = res.exec_time_ns
    LAST_RUN_INFO["profile_json"] = getattr(res, "profile_json", None)

    # ---- reassemble alpha [S, B, T]
    alpha = np.empty((S, B, T), dtype=np.float32)
    for i in range(NCORES):
        for c in range(2):
            arr = res.results[i][f"alpha_c{c}"]           # [128, S*16]
            a = arr.reshape(NQ, B_CHAIN, S, NN).transpose(2, 1, 0, 3) \
                   .reshape(S, B_CHAIN, T)
            b0 = i * B_CORE + c * B_CHAIN
            alpha[:, b0:b0 + B_CHAIN, :] = a

    # ---- host: terminal + exact backtrack (bitwise identical to reference)
    terminal = alpha[S - 1] + trans[END_IX][None, :]       # [B, T]
    path_score = terminal.max(axis=1)
    cur = terminal.argmax(axis=1).astype(np.int32)          # best_tag [B]
    best_path = np.empty((B, S), dtype=np.int32)
    best_path[:, S - 1] = cur
    for s in range(S - 1, 0, -1):
        nv = alpha[s - 1] + trans[cur, :]                   # [B, T]
        cur = nv.argmax(axis=1).astype(np.int32)
        best_path[:, s - 1] = cur
    return path_score, best_path


# revision 6
# speedup vs baseline: 1.6305x; 1.6305x over previous
"""CRF Viterbi decode kernel for Trainium2 (8 NeuronCores, data-parallel).

Device computes the forward max-plus scan (alpha) exactly; host does the
O(B*S*T) backtrack from alpha, which is bitwise-identical to the reference's
backpointer path.

Per core (64 batches), two independent 32-batch "chains" interleave so PE
matmuls of one chain overlap DVE reductions of the other:
  partition p = q*32 + b   (q in [0,4) tag-quarter, b in [0,32) batch)
  alpha slot s: [128, 16] tile region, value alpha_s[b, 16q+n]
  X psum [128, 1024]: X[(q,b), prev*16+n] = alpha_{s-1}[b, prev] + trans[16q+n, prev]
    - 4 selector matmuls (lhsT sel[qp], K=128) gather alpha_{s-1} (exact 0/1 weights)
    - 1 trans matmul (lhsT A4, K=4, rhs T4) accumulates the transition term
  DVE grouped reduce over prev (stride 16) -> m [128,16]; TT += feat -> alpha_s
"""
import os
import numpy as np

B, S, T = 512, 1024, 64
NCORES = 8
B_CORE = B // NCORES          # 64
B_CHAIN = B_CORE // 2         # 32
NQ, NN = 4, 16                # tag split: t = 16*q + n
CHUNK = 128                   # steps per feat/alpha chunk
NCHUNK = S // CHUNK
NEG = -10000.0
START_IX, END_IX = 62, 63

_CACHE = {}

LAST_RUN_INFO = {}


def _build_nc(trace=False):
    import concourse.bacc as bacc
    import concourse.tile as tile
    from concourse import mybir

    dt = mybir.dt.float32
    nc = bacc.Bacc()

    sel_d = [nc.declare_dram_parameter(f"sel{qp}", [128, 128], dt, isOutput=False)
             for qp in range(NQ)]
    A4_d = nc.declare_dram_parameter("A4", [NQ, 128], dt, isOutput=False)
    T4_d = nc.declare_dram_parameter("T4", [NQ, T * NN], dt, isOutput=False)
    feat_d = [nc.declare_dram_parameter(f"feat_c{c}", [128, S * NN], dt, isOutput=False)
              for c in range(2)]
    alpha_d = [nc.declare_dram_parameter(f"alpha_c{c}", [128, S * NN], dt, isOutput=True)
               for c in range(2)]

    with tile.TileContext(nc) as tc:
        with (
            tc.tile_pool(name="stage", bufs=1) as stage,
            tc.tile_pool(name="consts", bufs=1) as consts,
            tc.tile_pool(name="feat", bufs=2) as fpool,
            tc.tile_pool(name="alpha", bufs=2) as apool,
            tc.tile_pool(name="psum", bufs=1, space="PSUM") as pp,
        ):
            # ---- stage constants through DVE so PE waits only on the DVE sem
            sel_s = [stage.tile([128, 128], dt, name=f"sel_s{qp}", tag=f"sel_s{qp}") for qp in range(NQ)]
            A4_s = stage.tile([NQ, 128], dt, name="A4_s")
            T4_s = stage.tile([NQ, T * NN], dt, name="T4_s")
            for qp in range(NQ):
                nc.sync.dma_start(sel_s[qp][:], sel_d[qp][:])
            nc.sync.dma_start(A4_s[:], A4_d[:])
            nc.sync.dma_start(T4_s[:], T4_d[:])

            sel_t = [consts.tile([128, 128], dt, name=f"sel_t{qp}", tag=f"sel_t{qp}") for qp in range(NQ)]
            A4_t = consts.tile([NQ, 128], dt, name="A4_t")
            T4_t = consts.tile([NQ, T * NN], dt, name="T4_t")
            for qp in range(NQ):
                nc.vector.tensor_copy(sel_t[qp][:], sel_s[qp][:])
            nc.vector.tensor_copy(A4_t[:], A4_s[:])
            nc.vector.tensor_copy(T4_t[:], T4_s[:])

            # ---- HAM keepalive: fp32(HI/LO) matmuls don't count as PE-busy,
            # so the clock gate stays at 1.2GHz. A tiny bf16 matmul per step
            # keeps the activity window busy -> 2.4GHz for everything.
            bf = mybir.dt.bfloat16
            ka_w = consts.tile([32, 32], bf, name="ka_w")
            ka_x = consts.tile([32, 64], bf, name="ka_x")
            nc.vector.memset(ka_w[:], 0.0)
            nc.vector.memset(ka_x[:], 0.0)

            # ---- alpha_{-1} init: NEG everywhere, 0.0 at tag 62 = (q=3, n=14)
            init_t = [consts.tile([128, NN], dt, name=f"init{c}", tag=f"init{c}") for c in range(2)]
            for c in range(2):
                nc.vector.memset(init_t[c][:], NEG)
                nc.vector.memset(init_t[c][3 * 32:4 * 32, 14:15], 0.0)

            feat_tiles = [None, None]
            alpha_tiles = [None, None]
            prev_alpha = [None, None]   # (tile, col) of slot s-1

            # warmup burst: ~60us of continuous bf16 matmul activity so the
            # HAM sees busy SHORT windows and lifts the clock gate to 2.4GHz
            ka_ps = pp.tile([32, 64], mybir.dt.float32, name="ka_ps", tag="ka")
            for w in range(150):
                nc.tensor.matmul(ka_ps[:], ka_w[:], ka_x[:], start=True, stop=True)
            for s in range(S):
                k, j = divmod(s, CHUNK)
                if j == 0:
                    for c in range(2):
                        ft = fpool.tile([128, CHUNK * NN], dt, name=f"feat{c}_{k}", tag=f"feat{c}")
                        nc.sync.dma_start(
                            ft[:], feat_d[c][:, k * CHUNK * NN:(k + 1) * CHUNK * NN])
                        feat_tiles[c] = ft
                        alpha_tiles[c] = apool.tile([128, CHUNK * NN], dt, name=f"alpha{c}_{k}", tag=f"alpha{c}")
                for c in range(2):
                    at = alpha_tiles[c]
                    if s == 0:
                        slot_prev = init_t[c][:]
                    else:
                        pt, pj = prev_alpha[c]
                        slot_prev = pt[:, pj * NN:(pj + 1) * NN]
                    rhs = slot_prev.unsqueeze(2).broadcast_to([128, NN, NN])
                    X = pp.tile([128, T * NN], dt, name=f"X{c}_{s}", tag=f"X{c}")
                    # one start=True per PSUM bank: start clears has_written
                    # for the WHOLE bank, so the second mm in a bank must not
                    # re-clear (it overwrites where unset, which is its cols)
                    for qp in range(NQ):
                        nc.tensor.matmul(X[:, qp * 256:(qp + 1) * 256], sel_t[qp][:],
                                         rhs, start=(qp % 2 == 0), stop=False)
                    nc.tensor.matmul(X[:, 0:512], A4_t[:], T4_t[:, 0:512],
                                     start=False, stop=True)
                    nc.tensor.matmul(X[:, 512:1024], A4_t[:], T4_t[:, 512:1024],
                                     start=False, stop=True)
                    xin = X[:].rearrange("p (prev n) -> p n prev", n=NN)
                    m_ap = at[:, j * NN:(j + 1) * NN]
                    nc.vector.tensor_reduce(m_ap, xin, axis=mybir.AxisListType.X,
                                            op=mybir.AluOpType.max)
                    nc.vector.tensor_tensor(m_ap, m_ap,
                                            feat_tiles[c][:, j * NN:(j + 1) * NN],
                                            op=mybir.AluOpType.add)
                    prev_alpha[c] = (at, j)
                if j == CHUNK - 1:
                    for c in range(2):
                        nc.sync.dma_start(
                            alpha_d[c][:, k * CHUNK * NN:(k + 1) * CHUNK * NN],
                            alpha_tiles[c][:])

    nc.compile()
    return nc


def _host_inputs(feats, transitions):
    """Per-core input maps: selectors, A4, T4 layout of trans, per-chain feats."""
    feats = np.ascontiguousarray(np.asarray(feats, dtype=np.float32))
    trans = np.ascontiguousarray(np.asarray(transitions, dtype=np.float32))

    sel = [np.zeros((128, 128), dtype=np.float32) for _ in range(NQ)]
    for qp in range(NQ):
        for b in range(B_CHAIN):
            for q in range(NQ):
                sel[qp][qp * 32 + b, q * 32 + b] = 1.0
    A4 = np.zeros((NQ, 128), dtype=np.float32)
    for q in range(NQ):
        A4[q, q * 32:(q + 1) * 32] = 1.0
    # T4[q', prev*16+n] = trans[16*q'+n, prev]
    T4 = trans.reshape(NQ, NN, T).transpose(0, 2, 1).reshape(NQ, T * NN).copy()

    in_maps = []
    for i in range(NCORES):
        m = {f"sel{qp}": sel[qp] for qp in range(NQ)}
        m["A4"] = A4
        m["T4"] = T4
        for c in range(2):
            b0 = i * B_CORE + c * B_CHAIN
            blk = feats[b0:b0 + B_CHAIN]            # [32, S, 64]
            # row q*32+b, col s*16+n
            m[f"feat_c{c}"] = np.ascontiguousarray(
                blk.reshape(B_CHAIN, S, NQ, NN).transpose(2, 0, 1, 3)
                   .reshape(128, S * NN))
        in_maps.append(m)
    return in_maps, trans


def kernel(feats, transitions):
    from concourse.bass_utils import run_bass_kernel_spmd

    trace = bool(os.environ.get("CRF_KERNEL_TRACE"))
    key = "nc"
    if key not in _CACHE:
        _CACHE[key] = _build_nc()
    nc = _CACHE[key]

    in_maps, trans = _host_inputs(feats, transitions)
    res = run_bass_kernel_spmd(nc, in_maps, list(range(NCORES)), trace=trace)
    LAST_RUN_INFO["exec_time_ns"] = res.exec_time_ns
    LAST_RUN_INFO["profile_json"] = getattr(res, "profile_json", None)

    # ---- reassemble alpha [S, B, T]
    alpha = np.empty((S, B, T), dtype=np.float32)
    for i in range(NCORES):
        for c in range(2):
            arr = res.results[i][f"alpha_c{c}"]           # [128, S*16]
            a = arr.reshape(NQ, B_CHAIN, S, NN).transpose(2, 1, 0, 3) \
                   .reshape(S, B_CHAIN, T)
            b0 = i * B_CORE + c * B_CHAIN
            alpha[:, b0:b0 + B_CHAIN, :] = a

    # ---- host: terminal + exact backtrack (bitwise identical to reference)
    terminal = alpha[S - 1] + trans[END_IX][None, :]       # [B, T]
    path_score = terminal.max(axis=1)
    cur = terminal.argmax(axis=1).astype(np.int32)          # best_tag [B]
    best_path = np.empty((B, S), dtype=np.int32)
    best_path[:, S - 1] = cur
    for s in range(S - 1, 0, -1):
        nv = alpha[s - 1] + trans[cur, :]                   # [B, T]
        cur = nv.argmax(axis=1).astype(np.int32)
        best_path[:, s - 1] = cur
    return path_score, best_path


# revision 7
# speedup vs baseline: 1.8330x; 1.1242x over previous
"""CRF Viterbi decode kernel for Trainium2 (8 NeuronCores, data-parallel).

Device computes the forward max-plus scan (alpha) exactly; host does the
O(B*S*T) backtrack from alpha, which is bitwise-identical to the reference's
backpointer path.

Per core (64 batches), two independent 32-batch "chains" interleave so PE
matmuls of one chain overlap DVE reductions of the other:
  partition p = q*32 + b   (q in [0,4) tag-quarter, b in [0,32) batch)
  alpha slot s: [128, 16] tile region, value alpha_s[b, 16q+n]
  X psum [128, 1024]: X[(q,b), prev*16+n] = alpha_{s-1}[b, prev] + trans[16q+n, prev]
    - 4 selector matmuls (lhsT sel[qp], K=128) gather alpha_{s-1} (exact 0/1 weights)
    - 1 trans matmul (lhsT A4, K=4, rhs T4) accumulates the transition term
  DVE grouped reduce over prev (stride 16) -> m [128,16]; TT += feat -> alpha_s
"""
import os
import numpy as np

B, S, T = 512, 1024, 64
NCORES = 8
B_CORE = B // NCORES          # 64
B_CHAIN = B_CORE // 2         # 32
NQ, NN = 4, 16                # tag split: t = 16*q + n
CHUNK = 128                   # steps per feat/alpha chunk
NCHUNK = S // CHUNK
NEG = -10000.0
START_IX, END_IX = 62, 63

_CACHE = {}

LAST_RUN_INFO = {}


def _build_nc(trace=False):
    import concourse.bacc as bacc
    import concourse.tile as tile
    from concourse import mybir

    dt = mybir.dt.float32
    nc = bacc.Bacc()

    sel_d = [nc.declare_dram_parameter(f"sel{qp}", [128, 128], dt, isOutput=False)
             for qp in range(NQ)]
    T4B_d = nc.declare_dram_parameter("T4B", [128, T * NN], dt, isOutput=False)
    feat_d = [nc.declare_dram_parameter(f"feat_c{c}", [128, S * NN], dt, isOutput=False)
              for c in range(2)]
    alpha_d = [nc.declare_dram_parameter(f"alpha_c{c}", [128, S * NN], dt, isOutput=True)
               for c in range(2)]

    with tile.TileContext(nc) as tc:
        with (
            tc.tile_pool(name="stage", bufs=1) as stage,
            tc.tile_pool(name="consts", bufs=1) as consts,
            tc.tile_pool(name="feat", bufs=2) as fpool,
            tc.tile_pool(name="alpha", bufs=2) as apool,
            tc.tile_pool(name="ypool", bufs=2) as ypool,
            tc.tile_pool(name="psum", bufs=1, space="PSUM") as pp,
        ):
            # ---- stage constants through DVE so PE waits only on the DVE sem
            sel_s = [stage.tile([128, 128], dt, name=f"sel_s{qp}", tag=f"sel_s{qp}") for qp in range(NQ)]
            for qp in range(NQ):
                nc.sync.dma_start(sel_s[qp][:], sel_d[qp][:])

            sel_t = [consts.tile([128, 128], dt, name=f"sel_t{qp}", tag=f"sel_t{qp}") for qp in range(NQ)]
            T4B_t = consts.tile([128, T * NN], dt, name="T4B_t")
            for qp in range(NQ):
                nc.vector.tensor_copy(sel_t[qp][:], sel_s[qp][:])
            nc.sync.dma_start(T4B_t[:], T4B_d[:])

            # ---- alpha_{-1} init: NEG everywhere, 0.0 at tag 62 = (q=3, n=14)
            init_t = [consts.tile([128, NN], dt, name=f"init{c}", tag=f"init{c}") for c in range(2)]
            for c in range(2):
                nc.vector.memset(init_t[c][:], NEG)
                nc.vector.memset(init_t[c][3 * 32:4 * 32, 14:15], 0.0)

            feat_tiles = [None, None]
            alpha_tiles = [None, None]
            prev_alpha = [None, None]   # (tile, col) of slot s-1

            ypool_tiles = [None, None]
            for s in range(S):
                k, j = divmod(s, CHUNK)
                if j == 0:
                    for c in range(2):
                        ft = fpool.tile([128, CHUNK * NN], dt, name=f"feat{c}_{k}", tag=f"feat{c}")
                        nc.sync.dma_start(
                            ft[:], feat_d[c][:, k * CHUNK * NN:(k + 1) * CHUNK * NN])
                        feat_tiles[c] = ft
                        alpha_tiles[c] = apool.tile([128, CHUNK * NN], dt, name=f"alpha{c}_{k}", tag=f"alpha{c}")
                for c in range(2):
                    at = alpha_tiles[c]
                    if s == 0:
                        slot_prev = init_t[c][:]
                    else:
                        pt, pj = prev_alpha[c]
                        slot_prev = pt[:, pj * NN:(pj + 1) * NN]
                    rhs = slot_prev.unsqueeze(2).broadcast_to([128, NN, NN])
                    X = pp.tile([128, T * NN], dt, name=f"X{c}_{s}", tag=f"X{c}")
                    # one start=True per PSUM bank: start clears has_written
                    # for the WHOLE bank, so the second mm in a bank must not
                    # re-clear (it overwrites where unset, which is its cols)
                    for qp in range(NQ):
                        nc.tensor.matmul(X[:, qp * 256:(qp + 1) * 256], sel_t[qp][:],
                                         rhs, start=(qp % 2 == 0), stop=(qp % 2 == 1))
                    # trans-add on DVE: Y = (X + 0.0) + T4B  (one rounding, exact)
                    Y = ypool.tile([128, T * NN], dt, name=f"Y{c}_{s}", tag=f"Y{c}")
                    nc.vector.scalar_tensor_tensor(Y[:], X[:], 0.0, T4B_t[:],
                                                   op0=mybir.AluOpType.add,
                                                   op1=mybir.AluOpType.add)
                    xin = Y[:].rearrange("p (prev n) -> p n prev", n=NN)
                    m_ap = at[:, j * NN:(j + 1) * NN]
                    nc.vector.tensor_reduce(m_ap, xin, axis=mybir.AxisListType.X,
                                            op=mybir.AluOpType.max)
                    nc.vector.tensor_tensor(m_ap, m_ap,
                                            feat_tiles[c][:, j * NN:(j + 1) * NN],
                                            op=mybir.AluOpType.add)
                    prev_alpha[c] = (at, j)
                if j == CHUNK - 1:
                    for c in range(2):
                        nc.sync.dma_start(
                            alpha_d[c][:, k * CHUNK * NN:(k + 1) * CHUNK * NN],
                            alpha_tiles[c][:])

    nc.compile()
    return nc


def _host_inputs(feats, transitions):
    """Per-core input maps: selectors, A4, T4 layout of trans, per-chain feats."""
    feats = np.ascontiguousarray(np.asarray(feats, dtype=np.float32))
    trans = np.ascontiguousarray(np.asarray(transitions, dtype=np.float32))

    sel = [np.zeros((128, 128), dtype=np.float32) for _ in range(NQ)]
    for qp in range(NQ):
        for b in range(B_CHAIN):
            for q in range(NQ):
                sel[qp][qp * 32 + b, q * 32 + b] = 1.0
    # T4[q', prev*16+n] = trans[16*q'+n, prev]; T4B replicates row q to rows q*32..q*32+32
    T4 = trans.reshape(NQ, NN, T).transpose(0, 2, 1).reshape(NQ, T * NN)
    T4B = np.ascontiguousarray(np.repeat(T4, B_CHAIN, axis=0))

    in_maps = []
    for i in range(NCORES):
        m = {f"sel{qp}": sel[qp] for qp in range(NQ)}
        m["T4B"] = T4B
        for c in range(2):
            b0 = i * B_CORE + c * B_CHAIN
            blk = feats[b0:b0 + B_CHAIN]            # [32, S, 64]
            # row q*32+b, col s*16+n
            m[f"feat_c{c}"] = np.ascontiguousarray(
                blk.reshape(B_CHAIN, S, NQ, NN).transpose(2, 0, 1, 3)
                   .reshape(128, S * NN))
        in_maps.append(m)
    return in_maps, trans


def kernel(feats, transitions):
    from concourse.bass_utils import run_bass_kernel_spmd

    trace = bool(os.environ.get("CRF_KERNEL_TRACE"))
    key = "nc"
    if key not in _CACHE:
        _CACHE[key] = _build_nc()
    nc = _CACHE[key]

    in_maps, trans = _host_inputs(feats, transitions)
    res = run_bass_kernel_spmd(nc, in_maps, list(range(NCORES)), trace=trace)
    LAST_RUN_INFO["exec_time_ns"] = res.exec_time_ns
    LAST_RUN_INFO["profile_json"] = getattr(res, "profile_json", None)

    # ---- reassemble alpha [S, B, T]
    alpha = np.empty((S, B, T), dtype=np.float32)
    for i in range(NCORES):
        for c in range(2):
            arr = res.results[i][f"alpha_c{c}"]           # [128, S*16]
            a = arr.reshape(NQ, B_CHAIN, S, NN).transpose(2, 1, 0, 3) \
                   .reshape(S, B_CHAIN, T)
            b0 = i * B_CORE + c * B_CHAIN
            alpha[:, b0:b0 + B_CHAIN, :] = a

    # ---- host: terminal + exact backtrack (bitwise identical to reference)
    terminal = alpha[S - 1] + trans[END_IX][None, :]       # [B, T]
    path_score = terminal.max(axis=1)
    cur = terminal.argmax(axis=1).astype(np.int32)          # best_tag [B]
    best_path = np.empty((B, S), dtype=np.int32)
    best_path[:, S - 1] = cur
    for s in range(S - 1, 0, -1):
        nv = alpha[s - 1] + trans[cur, :]                   # [B, T]
        cur = nv.argmax(axis=1).astype(np.int32)
        best_path[:, s - 1] = cur
    return path_score, best_path


# revision 8
# speedup vs baseline: 1.9862x; 1.0835x over previous
"""CRF Viterbi decode kernel for Trainium2 (8 NeuronCores, data-parallel).

Device computes the forward max-plus scan (alpha) exactly; host does the
O(B*S*T) backtrack from alpha, which is bitwise-identical to the reference's
backpointer path.

Per core (64 batches), two independent 32-batch "chains" interleave so PE
matmuls of one chain overlap DVE reductions of the other:
  partition p = q*32 + b   (q in [0,4) tag-quarter, b in [0,32) batch)
  alpha slot s: [128, 16] tile region, value alpha_s[b, 16q+n]
  X psum [128, 1024]: X[(q,b), prev*16+n] = alpha_{s-1}[b, prev] + trans[16q+n, prev]
    - 4 selector matmuls (lhsT sel[qp], K=128) gather alpha_{s-1} (exact 0/1 weights)
    - 1 trans matmul (lhsT A4, K=4, rhs T4) accumulates the transition term
  DVE grouped reduce over prev (stride 16) -> m [128,16]; TT += feat -> alpha_s
"""
import os
import numpy as np

B, S, T = 512, 1024, 64
NCORES = 8
B_CORE = B // NCORES          # 64
B_CHAIN = B_CORE // 2         # 32
NQ, NN = 4, 16                # tag split: t = 16*q + n
CHUNK = 128                   # steps per feat/alpha chunk
NCHUNK = S // CHUNK
NEG = -10000.0
START_IX, END_IX = 62, 63

_CACHE = {}

LAST_RUN_INFO = {}


def _build_nc(trace=False):
    import concourse.bacc as bacc
    import concourse.tile as tile
    from concourse import mybir

    dt = mybir.dt.float32
    nc = bacc.Bacc()

    sel_d = [nc.declare_dram_parameter(f"sel{qp}", [128, 128], dt, isOutput=False)
             for qp in range(NQ)]
    T4B_d = nc.declare_dram_parameter("T4B", [128, T * NN], dt, isOutput=False)
    feat_d = [nc.declare_dram_parameter(f"feat_c{c}", [128, S * NN], dt, isOutput=False)
              for c in range(2)]
    alpha_d = [nc.declare_dram_parameter(f"alpha_c{c}", [128, S * NN], dt, isOutput=True)
               for c in range(2)]

    with tile.TileContext(nc) as tc:
        with (
            tc.tile_pool(name="stage", bufs=1) as stage,
            tc.tile_pool(name="consts", bufs=1) as consts,
            tc.tile_pool(name="feat", bufs=2) as fpool,
            tc.tile_pool(name="alpha", bufs=2) as apool,
            tc.tile_pool(name="ypool", bufs=2) as ypool,
            tc.tile_pool(name="psum", bufs=1, space="PSUM") as pp,
        ):
            # ---- stage constants through DVE so PE waits only on the DVE sem
            sel_s = [stage.tile([128, 128], dt, name=f"sel_s{qp}", tag=f"sel_s{qp}") for qp in range(NQ)]
            for qp in range(NQ):
                nc.sync.dma_start(sel_s[qp][:], sel_d[qp][:])

            sel_t = [consts.tile([128, 128], dt, name=f"sel_t{qp}", tag=f"sel_t{qp}") for qp in range(NQ)]
            T4B_t = consts.tile([128, T * NN], dt, name="T4B_t")
            for qp in range(NQ):
                nc.vector.tensor_copy(sel_t[qp][:], sel_s[qp][:])
            nc.sync.dma_start(T4B_t[:], T4B_d[:])

            # ---- alpha_{-1} init: NEG everywhere, 0.0 at tag 62 = (q=3, n=14)
            init_t = [consts.tile([128, NN], dt, name=f"init{c}", tag=f"init{c}") for c in range(2)]
            for c in range(2):
                nc.vector.memset(init_t[c][:], NEG)
                nc.vector.memset(init_t[c][3 * 32:4 * 32, 14:15], 0.0)

            feat_tiles = [None, None]
            alpha_tiles = [None, None]
            prev_alpha = [None, None]   # (tile, col) of slot s-1

            ypool_tiles = [None, None]
            for s in range(S):
                k, j = divmod(s, CHUNK)
                if j == 0:
                    for c in range(2):
                        ft = fpool.tile([128, CHUNK * NN], dt, name=f"feat{c}_{k}", tag=f"feat{c}")
                        nc.sync.dma_start(
                            ft[:], feat_d[c][:, k * CHUNK * NN:(k + 1) * CHUNK * NN])
                        feat_tiles[c] = ft
                        alpha_tiles[c] = apool.tile([128, CHUNK * NN], dt, name=f"alpha{c}_{k}", tag=f"alpha{c}")
                for c in range(2):
                    at = alpha_tiles[c]
                    if s == 0:
                        slot_prev = init_t[c][:]
                    else:
                        pt, pj = prev_alpha[c]
                        slot_prev = pt[:, pj * NN:(pj + 1) * NN]
                    rhs = slot_prev.unsqueeze(2).broadcast_to([128, NN, NN])
                    X = pp.tile([128, T * NN], dt, name=f"X{c}_{s}", tag=f"X{c}")
                    # one start=True per PSUM bank: start clears has_written
                    # for the WHOLE bank, so the second mm in a bank must not
                    # re-clear (it overwrites where unset, which is its cols)
                    for qp in range(NQ):
                        nc.tensor.matmul(X[:, qp * 256:(qp + 1) * 256], sel_t[qp][:],
                                         rhs, start=(qp % 2 == 0), stop=(qp % 2 == 1))
                    # trans-add on DVE: Y = (X + 0.0) + T4B  (one rounding, exact)
                    # read X n-major (PSUM strides are free), write Y contiguous
                    # n-major so the reduce gets a contiguous innermost axis
                    Y = ypool.tile([128, T * NN], dt, name=f"Y{c}_{s}", tag=f"Y{c}")
                    nc.vector.scalar_tensor_tensor(
                        Y[:].rearrange("p (n prev) -> p n prev", n=NN),
                        X[:].rearrange("p (prev n) -> p n prev", n=NN),
                        0.0,
                        T4B_t[:].rearrange("p (n prev) -> p n prev", n=NN),
                        op0=mybir.AluOpType.add,
                        op1=mybir.AluOpType.add)
                    xin = Y[:].rearrange("p (n prev) -> p n prev", n=NN)
                    m_ap = at[:, j * NN:(j + 1) * NN]
                    nc.vector.tensor_reduce(m_ap, xin, axis=mybir.AxisListType.X,
                                            op=mybir.AluOpType.max)
                    nc.vector.tensor_tensor(m_ap, m_ap,
                                            feat_tiles[c][:, j * NN:(j + 1) * NN],
                                            op=mybir.AluOpType.add)
                    prev_alpha[c] = (at, j)
                if j == CHUNK - 1:
                    for c in range(2):
                        nc.sync.dma_start(
                            alpha_d[c][:, k * CHUNK * NN:(k + 1) * CHUNK * NN],
                            alpha_tiles[c][:])

    nc.compile()
    return nc


def _host_inputs(feats, transitions):
    """Per-core input maps: selectors, A4, T4 layout of trans, per-chain feats."""
    feats = np.ascontiguousarray(np.asarray(feats, dtype=np.float32))
    trans = np.ascontiguousarray(np.asarray(transitions, dtype=np.float32))

    sel = [np.zeros((128, 128), dtype=np.float32) for _ in range(NQ)]
    for qp in range(NQ):
        for b in range(B_CHAIN):
            for q in range(NQ):
                sel[qp][qp * 32 + b, q * 32 + b] = 1.0
    # T4B row q*32+b, col n*64+prev = trans[16*q+n, prev] (n-major)
    T4 = trans.reshape(NQ, NN, T).reshape(NQ, T * NN)
    T4B = np.ascontiguousarray(np.repeat(T4, B_CHAIN, axis=0))

    in_maps = []
    for i in range(NCORES):
        m = {f"sel{qp}": sel[qp] for qp in range(NQ)}
        m["T4B"] = T4B
        for c in range(2):
            b0 = i * B_CORE + c * B_CHAIN
            blk = feats[b0:b0 + B_CHAIN]            # [32, S, 64]
            # row q*32+b, col s*16+n
            m[f"feat_c{c}"] = np.ascontiguousarray(
                blk.reshape(B_CHAIN, S, NQ, NN).transpose(2, 0, 1, 3)
                   .reshape(128, S * NN))
        in_maps.append(m)
    return in_maps, trans


def kernel(feats, transitions):
    from concourse.bass_utils import run_bass_kernel_spmd

    trace = bool(os.environ.get("CRF_KERNEL_TRACE"))
    key = "nc"
    if key not in _CACHE:
        _CACHE[key] = _build_nc()
    nc = _CACHE[key]

    in_maps, trans = _host_inputs(feats, transitions)
    res = run_bass_kernel_spmd(nc, in_maps, list(range(NCORES)), trace=trace)
    LAST_RUN_INFO["exec_time_ns"] = res.exec_time_ns
    LAST_RUN_INFO["profile_json"] = getattr(res, "profile_json", None)

    # ---- reassemble alpha [S, B, T]
    alpha = np.empty((S, B, T), dtype=np.float32)
    for i in range(NCORES):
        for c in range(2):
            arr = res.results[i][f"alpha_c{c}"]           # [128, S*16]
            a = arr.reshape(NQ, B_CHAIN, S, NN).transpose(2, 1, 0, 3) \
                   .reshape(S, B_CHAIN, T)
            b0 = i * B_CORE + c * B_CHAIN
            alpha[:, b0:b0 + B_CHAIN, :] = a

    # ---- host: terminal + exact backtrack (bitwise identical to reference)
    terminal = alpha[S - 1] + trans[END_IX][None, :]       # [B, T]
    path_score = terminal.max(axis=1)
    cur = terminal.argmax(axis=1).astype(np.int32)          # best_tag [B]
    best_path = np.empty((B, S), dtype=np.int32)
    best_path[:, S - 1] = cur
    for s in range(S - 1, 0, -1):
        nv = alpha[s - 1] + trans[cur, :]                   # [B, T]
        cur = nv.argmax(axis=1).astype(np.int32)
        best_path[:, s - 1] = cur
    return path_score, best_path
